# revision 33
# baseline (speedup 1.0000x reference)
"""AttentionBlock (InstanceNorm + single-head self-attention over 64x64 pixels
+ residual) on 8 Trainium2 NeuronCores.

Sharding: core = (batch b = core//2, query-half h = core%2). Each core gets the
full 512x4096 plane of its batch (columns rolled so its 2048 query pixels are
columns 0..2047), computes norm + K/V for all 4096 pixels and Q for its 2048,
runs softmax(Q^T K / sqrt(C)) V and the output projection for its half, and
returns a [512, 2048] shard. No collectives.

All five matmul stages run as fp8(e4m3) DoubleRow matmuls (256-deep
contraction per instruction) with fp32 PSUM accumulation:
 - x ships from host already in fp8 DR pair layout (2 planes [128,2,4096]);
   a bf16 copy of the query half ships separately for the residual add.
 - InstanceNorm is folded into the projections: q/k/v = (W*rstd)@x + corr.
   The scaled weights are built on-device as fp8 at 16x scale (ws=W*16*rstd)
   to stay out of fp8 denormals; q/k/v PSUM results are then 16x the true
   values, which cancels exactly through the scale-invariant softmax
   (QK logits carry 1/256 in the exp scale; the 16x on V and the 4x on the
   wp weights are divided out via the softmax denominator: the Z reduction
   uses ones of value 64 = 16*4).
 - the InstanceNorm bias corrections W@(-mu*rstd) run as tiny fp8 DR matmuls
   of the scaled weights against an fp8 (-16*mu) vector, with the epilogue
   dividing the scales back out. No bf16 weight matmuls remain.
 - exp() offset -4.25 keeps fp8 exp in (0, 240); it cancels in U/Z.
 - o = U/Z is normalized BEFORE the output projection (fp8 at v/4 scale), so
   the P projection also runs fp8 DR.
The softmax denominator Z accumulates on DVE (adds over the fp8 exp tiles);
a single bf16 ones-matmul reduces it across partitions and broadcasts it into
a PSUM bank riding the QK-pair rotation. exp() runs one ACT instruction per
psL pair (two PSUM banks) to amortize instruction overhead. The first EV
accumulation pair of each i-block after the first is deferred into the middle
of the j-loop so the in-order PE never stalls on the previous block's
epilogue reads of the psU banks (which double as the output-projection PSUM).
Output is written bf16 and upcast on host.
"""

import numpy as np
import ml_dtypes

import concourse.bass as bass
import concourse.mybir as mybir
import concourse.tile as tile
from concourse import bacc
from concourse import bass_utils

C = 512          # channels
HW = 4096        # pixels per plane (64*64)
NQ = 2048        # query pixels per core
B = 4            # batch
N_CORES = 8
CT = C // 128    # channel tiles (4)
JT = HW // 128   # key tiles on partitions (32)
JP = JT // 2     # key tile pairs for DoubleRow (16)
IB = NQ // 512   # query i-blocks of 512 (4)
KNB = HW // 512  # key n-chunks for k projection (8)
EPS = 1e-5
SCALE = 1.0 / np.sqrt(np.float32(C))  # 1/sqrt(512)
WS = 16.0        # ws weight scale (q/k/v psum = 16x true)
ALPHA = 4.0      # wp weight scale (o stored as v/4)
BETA = WS * ALPHA  # Z-reduction ones value (cancels WS on V, ALPHA on wp)
EXP_OFF = -4.25  # exp offset; cancels in U/Z, keeps fp8 exp in range

F32 = mybir.dt.float32
BF16 = mybir.dt.bfloat16
FP8 = mybir.dt.float8e4
AF = mybir.ActivationFunctionType
ALU = mybir.AluOpType


def build_nc():
    nc = bacc.Bacc("TRN2", target_bir_lowering=False, debug=False,
                   num_devices=N_CORES)
    # x in fp8 DR pair layout: x8[p, 2g+r, j] = x[(2g+r)*128 + p, j]
    x8 = nc.dram_tensor("x8", [128, CT, HW], FP8, kind="ExternalInput").ap()
    # bf16 residual (query half, raw x)
    xr = nc.dram_tensor("xr", [C, NQ], BF16, kind="ExternalInput").ap()
    # packed weights: w_rs[p, ct*C + o] = w.T[ct*128 + p, o]
    wqT = nc.dram_tensor("wqT", [128, CT * C], BF16, kind="ExternalInput").ap()
    wkT = nc.dram_tensor("wkT", [128, CT * C], BF16, kind="ExternalInput").ap()
    wvT = nc.dram_tensor("wvT", [128, CT * C], BF16, kind="ExternalInput").ap()
    wpT = nc.dram_tensor("wpT", [128, CT * C], BF16, kind="ExternalInput").ap()
    # packed bias constants [128, 12]: cols 0-3 = 16*bq, 4-7 = 16*bk,
    # 8-11 = wp@bv + bp, laid out b[ct*128 + p] -> [p, ct]
    bcons = nc.dram_tensor("bcons", [128, 3 * CT], F32,
                           kind="ExternalInput").ap()
    out = nc.dram_tensor("out", [C, NQ], BF16, kind="ExternalOutput").ap()

    with tile.TileContext(nc) as tc:
        build_graph(tc, x8, xr, wqT, wkT, wvT, wpT, bcons, out)
    nc.compile()
    return nc


def build_graph(tc, x8, xr, wqT, wkT, wvT, wpT, bcons, out):
    nc = tc.nc
    DR = mybir.MatmulPerfMode.DoubleRow
    with (
        tc.tile_pool(name="const", bufs=1) as const,
        tc.tile_pool(name="qk", bufs=1) as qkp,
        tc.tile_pool(name="vt", bufs=1) as vtp,
        tc.tile_pool(name="bc", bufs=1) as bcp,
    ):
        # ---- persistent tiles ----
        x_sb = [qkp.tile([128, 2, HW], FP8, tag=f"x{g}", name=f"x{g}")
                for g in range(2)]
        q_sb = [qkp.tile([128, 2, NQ], FP8, tag=f"q{g}", name=f"q{g}")
                for g in range(2)]
        k_sb = [qkp.tile([128, 2, HW], FP8, tag=f"k{g}", name=f"k{g}")
                for g in range(2)]
        vT_sb = [vtp.tile([128, 2, C], FP8, tag=f"vT{jtp}", name=f"vT{jtp}")
                 for jtp in range(JP)]
        ws_sb = {}
        for wname in ("wq", "wk", "wv"):
            ws_sb[wname] = [qkp.tile([128, 2, C], FP8, tag=f"{wname}s{g}",
                                     name=f"{wname}s{g}") for g in range(2)]
        wp4_sb = [qkp.tile([128, 2, C], FP8, tag=f"wp4{g}", name=f"wp4{g}")
                  for g in range(2)]

        # ---- x first, split across both hwdge trigger queues so neither
        # serializes: sync takes the DVE stats planes (0,1,2h1), scalar takes
        # the ACT stats planes (3, 2h2) ----
        x_dma_insts = []
        for ct, h, eng in ((0, 0, nc.sync), (3, 0, nc.scalar),
                           (0, 1, nc.sync), (3, 1, nc.scalar),
                           (1, 0, nc.sync), (1, 1, nc.sync),
                           (2, 0, nc.sync), (2, 1, nc.sync)):
            g, r = divmod(ct, 2)
            csl = slice(h * 2048, (h + 1) * 2048)
            x_dma_insts.append(eng.dma_start(
                out=x_sb[g][:, r, csl], in_=x8[:, ct, csl]))
        # weights + bias constants on the sync queue
        w_dma_insts = []
        w_sb = {}
        for wname, wap in (("wq", wqT), ("wk", wkT), ("wv", wvT), ("wp", wpT)):
            t = const.tile([128, CT * C], BF16, tag=wname, name=wname)
            w_dma_insts.append(nc.sync.dma_start(out=t, in_=wap))
            w_sb[wname] = [t[:, ct * C:(ct + 1) * C] for ct in range(CT)]
        bct = const.tile([128, 3 * CT], F32, tag="bcons", name="bcons")
        nc.scalar.dma_start(out=bct, in_=bcons)
        bq_sb = [bct[:, ct:ct + 1] for ct in range(CT)]
        bk_sb = [bct[:, CT + ct:CT + ct + 1] for ct in range(CT)]
        bp2_sb = [bct[:, 2 * CT + ct:2 * CT + ct + 1] for ct in range(CT)]
        ones_sb = const.tile([128, 128], BF16, tag="ones", name="ones")
        nc.vector.memset(ones_sb, BETA)
        eps_sb = const.tile([128, 1], F32, tag="eps", name="eps")
        nc.vector.memset(eps_sb, EPS / (WS * WS))
        expoff_sb = const.tile([128, 1], F32, tag="expoff", name="expoff")
        nc.vector.memset(expoff_sb, EXP_OFF)
        # preload both ACT function tables while ACT waits on x (Square lives
        # in table 0; Sqrt/Exp in table 1 - the lazy load would otherwise put
        # a 1.3us ACT_TABLE_LOAD on the stats critical path)
        dummy_sb = const.tile([128, 1], F32, tag="dummy", name="dummy")
        nc.scalar.activation(out=dummy_sb, in_=eps_sb, func=AF.Sqrt)

        # bias-correction result tiles (written by tiny matmuls below)
        qbias_sb = [bcp.tile([128, 1], F32, tag=f"qb{mt}", name=f"qb{mt}")
                    for mt in range(CT)]
        kbias_sb = [bcp.tile([128, 1], F32, tag=f"kb{mt}", name=f"kb{mt}")
                    for mt in range(CT)]
        # [128, 2, 16] with data in col 0: DoubleRow needs pair stride %16==0
        cvn_sb = [bcp.tile([128, 2, 16], FP8, tag=f"cvn{g}", name=f"cvn{g}")
                  for g in range(2)]
        wpcv_sb = [bcp.tile([128, 1], F32, tag=f"wpcv{mt}", name=f"wpcv{mt}")
                   for mt in range(CT)]
        nmbx_sb = [bcp.tile([128, 2, 16], FP8, tag=f"nmbx{g}", name=f"nmbx{g}")
                   for g in range(2)]

        def q_epi_dst(ct2, nsl):
            return q_sb[ct2 // 2][:, ct2 % 2, nsl]

        def k_epi_dst(ct2, nsl):
            return k_sb[ct2 // 2][:, ct2 % 2, nsl]

        def vt_epi_dst(jt):
            return vT_sb[jt // 2][:, jt % 2, :]

        with (
            tc.tile_pool(name="stat", bufs=4) as stat,
            tc.tile_pool(name="psB", bufs=6, space="PSUM") as psB,
        ):
            # ---- stage A: InstanceNorm stats -> rstd16/nmbx; fp8 weights ----
            # mv4[:, ct, 0] = mu, mv4[:, ct, 1] = var
            mv4 = stat.tile([128, CT, 2], F32, tag="mv4", name="mv4", bufs=1)
            scratch = stat.tile([128, 2048], BF16, tag="scratch",
                                name="scratch", bufs=1)
            # ACT: plane 3 via accumulating Copy (sums) then Square (sum of
            # squares). Copies first - Square's table preloads meanwhile.
            x_p3 = x_sb[1][:, 1, :]
            sx3 = stat.tile([128, 2], F32, tag="sx3", name="sx3", bufs=1)
            sq3 = stat.tile([128, 2], F32, tag="sq3", name="sq3", bufs=1)
            sc2 = stat.tile([128, 2048], BF16, tag="scratch2",
                            name="scratch2", bufs=1)
            nc.scalar.activation(out=scratch, in_=x_p3[:, :2048], func=AF.Copy,
                                 accum_out=sx3[:, 0:1])
            nc.scalar.activation(out=sc2, in_=x_p3[:, 2048:], func=AF.Copy,
                                 accum_out=sx3[:, 1:2])
            nc.scalar.activation(out=scratch, in_=x_p3[:, :2048],
                                 func=AF.Square, accum_out=sq3[:, 0:1])
            nc.scalar.activation(out=sc2, in_=x_p3[:, 2048:], func=AF.Square,
                                 accum_out=sq3[:, 1:2])
            # DVE: planes 0,1,2 via bn_stats
            for ct in (0, 1, 2):
                g, r = divmod(ct, 2)
                x_pl = x_sb[g][:, r, :]
                stats = stat.tile([128, 8, 6], F32, tag="stats",
                                  name=f"stats{ct}", bufs=2)
                for sg in range(8):
                    nc.vector.bn_stats(out=stats[:, sg, :],
                                       in_=x_pl[:, sg * 512:(sg + 1) * 512])
                nc.vector.bn_aggr(out=mv4[:, ct, :], in_=stats)
            # combine plane 3
            s3 = stat.tile([128, 2], F32, tag="s3", name="s3", bufs=1)
            nc.vector.tensor_add(s3[:, 0:1], sx3[:, 0:1], sx3[:, 1:2])
            nc.vector.tensor_add(s3[:, 1:2], sq3[:, 0:1], sq3[:, 1:2])
            nc.vector.tensor_scalar_mul(mv4[:, 3, 0:1], s3[:, 0:1], 1.0 / HW)
            m3sq = stat.tile([128, 2], F32, tag="m3sq", name="m3sq", bufs=1)
            nc.vector.tensor_mul(m3sq[:, 0:1], mv4[:, 3, 0:1], mv4[:, 3, 0:1])
            nc.vector.tensor_scalar_mul(m3sq[:, 1:2], s3[:, 1:2], 1.0 / HW)
            nc.vector.tensor_sub(mv4[:, 3, 1:2], m3sq[:, 1:2], m3sq[:, 0:1])

            # rstd16 = 16/sqrt(var+eps), one packed op for all 4 ct
            std4 = stat.tile([128, CT], F32, tag="std4", name="std4", bufs=1)
            nc.scalar.activation(out=std4, in_=mv4[:, :, 1], func=AF.Sqrt,
                                 bias=eps_sb, scale=1.0 / (WS * WS))
            rstd4 = stat.tile([128, CT], F32, tag="rstd4", name="rstd4",
                              bufs=1)
            nc.vector.reciprocal(out=rstd4, in_=std4)
            # nmbx[g][:, r, 0] = -16*mu[2g+r] (fp8, feeds bias-corr matmuls)
            for g in range(2):
                nc.vector.tensor_scalar_mul(nmbx_sb[g][:, :, 0:1],
                                            mv4[:, 2 * g:2 * g + 2, 0:1],
                                            -WS)
            # warm the Exp table set (sqrt and exp live in different table
            # sets; its load would otherwise insert right before the first
            # attention exp and stall the QK->exp pipeline start by ~1.5us).
            # Reading std4 orders this AFTER the real Sqrt - the engine's
            # wait-queue lets ready instructions bypass blocked ones, so a
            # dep-free dummy would run early and get its table re-evicted.
            nc.scalar.activation(out=dummy_sb, in_=std4[:, 0:1], func=AF.Exp)

            # keep weights off the DMA queues until x has landed - they
            # otherwise steal HBM bandwidth from the startup-critical load
            for wi in w_dma_insts:
                bass._add_dep_helper(wi.ins, x_dma_insts[-1].ins, sync=True,
                                     reason="x load first")

            # fp8 DR weight tiles: ws[g][:, r, :] = w_sb[2g+r] * rstd16
            # (all on DVE - gpsimd tensor ops measure ~7.4us each on HW)
            for wname in ("wq", "wk", "wv"):
                for ct in range(CT):
                    g, r = divmod(ct, 2)
                    nc.vector.tensor_scalar_mul(ws_sb[wname][g][:, r, :],
                                                w_sb[wname][ct],
                                                rstd4[:, ct:ct + 1])
            for ct in range(CT):
                g, r = divmod(ct, 2)
                nc.vector.tensor_scalar_mul(wp4_sb[g][:, r, :],
                                            w_sb["wp"][ct], ALPHA)

            # ---- stage B: fp8 DR projections on x8 ----
            # Main matmul groups first, tiny bias-corr matmuls next, epilogues
            # last (epilogues read the bias tiles; the bias matmuls depend on
            # stats from every channel tile and would stall the PE if first).
            def corr_mms(wsname, dst_ps):
                # dst_ps[mt] = sum_c ws[c, mt*128..]* (-16*mu[c])
                for mt in range(CT):
                    psb = psB.tile([128, 1], F32, tag="psBb", bufs=2,
                                   name=f"ps_{wsname}c{mt}")
                    for g in range(2):
                        nc.tensor.matmul(
                            psb,
                            ws_sb[wsname][g][:, :, mt * 128:(mt + 1) * 128],
                            nmbx_sb[g][:, :, 0:1], start=(g == 0),
                            stop=(g == 1), perf_mode=DR)
                    dst_ps.append(psb)

            # q[ct2][:, n*512...] (only first NQ pixels)
            q_ps = {}
            for ct2 in range(CT):
                for n in range(IB):
                    nsl = slice(n * 512, (n + 1) * 512)
                    ps = psB.tile([128, 512], F32, tag="psB",
                                  name=f"psq{ct2}_{n}")
                    for g in range(2):
                        nc.tensor.matmul(
                            ps, ws_sb["wq"][g][:, :, ct2 * 128:(ct2 + 1) * 128],
                            x_sb[g][:, :, nsl],
                            start=(g == 0), stop=(g == 1), perf_mode=DR)
                    q_ps[(ct2, n)] = ps
            qc_ps = []
            corr_mms("wq", qc_ps)
            for mt in range(CT):
                # qbias = psb/16 + 16*bq   (psb = 256*(wq@nmb_true))
                nc.scalar.activation(out=qbias_sb[mt], in_=qc_ps[mt],
                                     func=AF.Identity, bias=bq_sb[mt],
                                     scale=1.0 / WS)
            for (ct2, n), ps in q_ps.items():
                nsl = slice(n * 512, (n + 1) * 512)
                nc.scalar.activation(
                    out=q_epi_dst(ct2, nsl), in_=ps, func=AF.Identity,
                    bias=qbias_sb[ct2], scale=1.0)
            # k[ct2] over all HW pixels
            k_ps = {}
            for ct2 in range(CT):
                for n in range(KNB):
                    nsl = slice(n * 512, (n + 1) * 512)
                    ps = psB.tile([128, 512], F32, tag="psB",
                                  name=f"psk{ct2}_{n}")
                    for g in range(2):
                        nc.tensor.matmul(
                            ps, ws_sb["wk"][g][:, :, ct2 * 128:(ct2 + 1) * 128],
                            x_sb[g][:, :, nsl],
                            start=(g == 0), stop=(g == 1), perf_mode=DR)
                    k_ps[(ct2, n)] = ps
            kc_ps = []
            corr_mms("wk", kc_ps)
            for mt in range(CT):
                nc.scalar.activation(out=kbias_sb[mt], in_=kc_ps[mt],
                                     func=AF.Identity, bias=bk_sb[mt],
                                     scale=1.0 / WS)
            for (ct2, n), ps in k_ps.items():
                nsl = slice(n * 512, (n + 1) * 512)
                if n % 2 == 0:
                    nc.scalar.activation(
                        out=k_epi_dst(ct2, nsl), in_=ps, func=AF.Identity,
                        bias=kbias_sb[ct2], scale=1.0)
                else:
                    nc.vector.tensor_scalar_add(k_epi_dst(ct2, nsl), ps,
                                                kbias_sb[ct2])
            # vT[jt] = [j=128, c=512] at 16x; v bias/const handled downstream
            for jt in range(JT):
                ps = psB.tile([128, 512], F32, tag="psB", name=f"psv{jt}")
                for g in range(2):
                    nc.tensor.matmul(
                        ps, x_sb[g][:, :, jt * 128:(jt + 1) * 128],
                        ws_sb["wv"][g],
                        start=(g == 0), stop=(g == 1), perf_mode=DR)
                if jt % 2 == 0:
                    nc.vector.tensor_copy(vt_epi_dst(jt), ps)
                else:
                    nc.scalar.activation(out=vt_epi_dst(jt), in_=ps,
                                         func=AF.Copy)
            # cvn = 128*(wv@nmb_true) as fp8 DR vector (= psb/2), then
            # wpcv = wp@cvn + bp2 via fp8 DR matmuls (psb = 512*wpcv)
            vc_ps = []
            corr_mms("wv", vc_ps)
            for mt in range(CT):
                # cvn8 = 128*(wv@nmb_true) = psb/2
                nc.scalar.activation(out=cvn_sb[mt // 2][:, mt % 2, 0:1],
                                     in_=vc_ps[mt], func=AF.Copy, scale=0.5)
            for mt in range(CT):
                psb = psB.tile([128, 1], F32, tag="psBb", bufs=2,
                               name=f"pswpcv{mt}")
                for g in range(2):
                    nc.tensor.matmul(
                        psb, wp4_sb[g][:, :, mt * 128:(mt + 1) * 128],
                        cvn_sb[g][:, :, 0:1], start=(g == 0), stop=(g == 1),
                        perf_mode=DR)
                nc.scalar.activation(out=wpcv_sb[mt], in_=psb,
                                     func=AF.Identity,
                                     bias=bp2_sb[mt], scale=2.0 / (256 * ALPHA))

        # ---- stage C: attention + output projection, per i-block ----
        with (
            tc.tile_pool(name="xres", bufs=16) as xresp,
            tc.tile_pool(name="expp", bufs=3) as expp,
            tc.tile_pool(name="exp0p", bufs=2) as exp0p,
            tc.tile_pool(name="op", bufs=2) as op,
            tc.tile_pool(name="yp", bufs=3) as yp,
            tc.tile_pool(name="rzp", bufs=2) as rzp,
            tc.tile_pool(name="zaccp", bufs=2) as zaccp,
            tc.tile_pool(name="psL", bufs=2, space="PSUM") as psLp,
            tc.tile_pool(name="psAcc", bufs=1, space="PSUM") as psAccp,
        ):
            first_exp_inst = None
            for ib in range(IB):
                isl = slice(ib * 512, (ib + 1) * 512)
                # defer the jtp=0 EV accumulation to the end of the j-loop on
                # later i-blocks: the EV chain then first writes psU ~2.7us
                # into the block, past the WAR on the previous block's
                # epilogue reads of the same banks (in-order PE would stall)
                defer = ib > 0
                psU = [psAccp.tile([128, 512], F32, tag=f"psU{ct}",
                                   name=f"psU{ct}_{ib}") for ct in range(CT)]
                zacc = zaccp.tile([128, 512], F32, tag="zacc", name=f"zacc{ib}")
                zaccb = zaccp.tile([128, 512], BF16, tag="zaccb",
                                   name=f"zaccb{ib}")

                # psL pair tiles (2 PSUM banks) -> one exp instruction per
                # pair. Software-pipelined: QK(jtp+1) issues before U(jtp).
                psL_pairs = [None] * JP
                exp_pairs = [None] * JP

                def emit_qk(jtp):
                    ps = psLp.tile([128, 2, 512], F32, tag="psL",
                                   name=f"psL{jtp}_{ib}")
                    for r in range(2):
                        jt = 2 * jtp + r
                        for g in range(2):
                            nc.tensor.matmul(
                                ps[:, r, :],
                                k_sb[g][:, :, jt * 128:(jt + 1) * 128],
                                q_sb[g][:, :, isl],
                                start=(g == 0), stop=(g == 1), perf_mode=DR)
                    psL_pairs[jtp] = ps

                def emit_ev(jtp, start, stop):
                    for ct in range(CT):
                        nc.tensor.matmul(
                            psU[ct], vT_sb[jtp][:, :, ct * 128:(ct + 1) * 128],
                            exp_pairs[jtp], start=start, stop=stop,
                            perf_mode=DR)

                emit_qk(0)
                for jtp in range(JP):
                    if jtp == 0 and defer:
                        ep = exp0p.tile([128, 2, 512], FP8, tag="expT0",
                                        name=f"expT0_{ib}")
                    else:
                        ep = expp.tile([128, 2, 512], FP8, tag="expT",
                                       name=f"expT{jtp}_{ib}")
                    exp_pairs[jtp] = ep
                    einst = nc.scalar.activation(
                        out=ep, in_=psL_pairs[jtp], func=AF.Exp,
                        bias=expoff_sb, scale=float(SCALE / (WS * WS)))
                    if first_exp_inst is None:
                        first_exp_inst = einst
                    if jtp + 1 < JP:
                        emit_qk(jtp + 1)
                    # Z reduction on DVE (half-pair granularity)
                    for r in range(2):
                        jt = 2 * jtp + r
                        exp_dst = ep[:, r, :]
                        if jt == 0:
                            nc.vector.tensor_copy(zacc, exp_dst)
                        elif jt == JT - 1:
                            nc.vector.tensor_add(zaccb, zacc, exp_dst)
                        else:
                            nc.vector.tensor_add(zacc, zacc, exp_dst)
                    if jtp > 0 or not defer:
                        emit_ev(jtp, start=(jtp == (1 if defer else 0)),
                                stop=(jtp == JP - 1))
                    if defer and jtp == 3:
                        # deferred pair-0 accumulation: far enough in that the
                        # previous block's epilogue has released the psU
                        # banks, and off this block's epilogue critical path
                        emit_ev(0, start=False, stop=False)

                # Z partition-reduce + broadcast in one bf16 matmul; psZ rides
                # the psL pair rotation (uses one of its two banks briefly)
                psZp = psLp.tile([128, 2, 512], F32, tag="psL",
                                 name=f"psZ{ib}")
                nc.tensor.matmul(psZp[:, 0, :], ones_sb, zaccb, start=True,
                                 stop=True)
                rzb = rzp.tile([128, 512], F32, tag="rzb", name=f"rzb{ib}")
                nc.vector.reciprocal_approx_fast(out=rzb, in_=psZp[:, 0, :])

                # normalize U before the projection -> fp8 o in DR layout
                o_sb = [op.tile([128, 2, 512], FP8, tag=f"o{g}",
                                name=f"o{g}_{ib}") for g in range(2)]
                for ct in range(CT):
                    nc.vector.tensor_mul(o_sb[ct // 2][:, ct % 2, :],
                                         psU[ct], rzb)

                # output projection (fp8 DR), then y = psP + wpcv_tot + xr.
                # psP reuses psU's bank for this mt: the WAR dependency (the
                # o normalize reading psU[mt]) coincides with psP's own data
                # dependency on o, so the in-order PE never stalls on it.
                for mt in range(CT):
                    psP = psAccp.tile([128, 512], F32, tag=f"psU{mt}",
                                      name=f"psP{mt}_{ib}")
                    for g in range(2):
                        nc.tensor.matmul(
                            psP, wp4_sb[g][:, :, mt * 128:(mt + 1) * 128],
                            o_sb[g],
                            start=(g == 0), stop=(g == 1), perf_mode=DR)
                    xrt = xresp.tile([128, 512], BF16, tag="xr",
                                     name=f"xr{mt}_{ib}")
                    xr_dma = nc.sync.dma_start(
                        out=xrt, in_=xr[mt * 128:(mt + 1) * 128, isl])
                    # keep the residual loads off the DMA queues until the
                    # attention stage is underway - they'd otherwise compete
                    # with the startup x load for HBM bandwidth
                    bass._add_dep_helper(xr_dma.ins, first_exp_inst.ins,
                                         sync=True,
                                         reason="delay residual load")
                    y = yp.tile([128, 512], BF16, tag="y", name=f"y{mt}_{ib}")
                    nc.vector.scalar_tensor_tensor(
                        out=y, in0=psP, scalar=wpcv_sb[mt], in1=xrt,
                        op0=ALU.add, op1=ALU.add)
                    # last block's output triggers ride the (by then idle)
                    # ACT queue - the sync queue's ~600ns/trigger would
                    # serialize into the kernel tail
                    oeng = nc.scalar if ib == IB - 1 else nc.sync
                    oeng.dma_start(out=out[mt * 128:(mt + 1) * 128, isl],
                                   in_=y)


_NC = None


def _get_nc():
    global _NC
    if _NC is None:
        _NC = build_nc()
    return _NC


def make_in_maps(x, wq, bq, wk, bk, wv, bv, wp, bp):
    x = np.asarray(x, dtype=np.float32)
    wq, wk, wv, wp = (np.asarray(a, dtype=np.float32) for a in (wq, wk, wv, wp))
    bq, bk, bv, bp = (np.asarray(a, dtype=np.float32) for a in (bq, bk, bv, bp))
    bp2 = wp @ bv + bp

    def pack_w(w):
        # [p, ct*C + o] = w.T[ct*128 + p, o]
        wT = np.ascontiguousarray(w.T)
        return np.ascontiguousarray(
            wT.reshape(CT, 128, C).transpose(1, 0, 2).reshape(128, CT * C)
        ).astype(ml_dtypes.bfloat16)

    def pack_b(b):
        return np.ascontiguousarray(b.reshape(CT, 128).T).astype(np.float32)

    bcons = np.concatenate(
        [pack_b(WS * bq), pack_b(WS * bk), pack_b(bp2)], axis=1)
    shared = {
        "wqT": pack_w(wq), "wkT": pack_w(wk), "wvT": pack_w(wv),
        "wpT": pack_w(wp), "bcons": np.ascontiguousarray(bcons),
    }
    in_maps = []
    for core in range(N_CORES):
        b, h = divmod(core, 2)
        xb = x[b].reshape(C, HW)
        xc = np.ascontiguousarray(np.roll(xb, -h * NQ, axis=1))
        # fp8 DR pair layout: x8[p, ct, j] = xc[ct*128 + p, j]
        x8 = np.ascontiguousarray(
            xc.reshape(CT, 128, HW).transpose(1, 0, 2)
        ).astype(ml_dtypes.float8_e4m3)
        in_maps.append({
            "x8": x8,
            "xr": xc[:, :NQ].astype(ml_dtypes.bfloat16),
            **shared,
        })
    return in_maps


def assemble_out(results):
    out = np.empty((B, C, HW), dtype=np.float32)
    for core in range(N_CORES):
        b, h = divmod(core, 2)
        out[b][:, h * NQ:(h + 1) * NQ] = results[core]["out"].astype(np.float32)
    return out.reshape(B, C, 64, 64)


def kernel(x, wq, bq, wk, bk, wv, bv, wp, bp):
    nc = _get_nc()
    in_maps = make_in_maps(x, wq, bq, wk, bk, wv, bv, wp, bp)
    res = bass_utils.run_bass_kernel_spmd(nc, in_maps,
                                          core_ids=list(range(N_CORES)))
    return assemble_out(res.results)


# revision 35
# speedup vs baseline: 1.0113x; 1.0113x over previous
"""AttentionBlock (InstanceNorm + single-head self-attention over 64x64 pixels
+ residual) on 8 Trainium2 NeuronCores.

Sharding: core = (batch b = core//2, query-half h = core%2). Each core gets the
full 512x4096 plane of its batch (columns rolled so its 2048 query pixels are
columns 0..2047), computes norm + K/V for all 4096 pixels and Q for its 2048,
runs softmax(Q^T K / sqrt(C)) V and the output projection for its half, and
returns a [512, 2048] shard. No collectives.

All five matmul stages run as fp8(e4m3) DoubleRow matmuls (256-deep
contraction per instruction) with fp32 PSUM accumulation:
 - x ships from host already in fp8 DR pair layout (2 planes [128,2,4096]);
   a bf16 copy of the query half ships separately for the residual add.
 - InstanceNorm is folded into the projections: q/k/v = (W*rstd)@x + corr.
   The scaled weights are built on-device as fp8 at 16x scale (ws=W*16*rstd)
   to stay out of fp8 denormals; q/k/v PSUM results are then 16x the true
   values, which cancels exactly through the scale-invariant softmax
   (QK logits carry 1/256 in the exp scale; the 16x on V and the 4x on the
   wp weights are divided out via the softmax denominator: the Z reduction
   uses ones of value 64 = 16*4).
 - the InstanceNorm bias corrections W@(-mu*rstd) run as tiny fp8 DR matmuls
   of the scaled weights against an fp8 (-16*mu) vector, with the epilogue
   dividing the scales back out. No bf16 weight matmuls remain.
 - exp() offset -4.25 keeps fp8 exp in (0, 240); it cancels in U/Z.
 - o = U/Z is normalized BEFORE the output projection (fp8 at v/4 scale), so
   the P projection also runs fp8 DR.
The softmax denominator Z accumulates on DVE (adds over the fp8 exp tiles);
a single bf16 ones-matmul reduces it across partitions and broadcasts it into
a PSUM bank riding the QK-pair rotation. exp() runs one ACT instruction per
psL pair (two PSUM banks) to amortize instruction overhead. The first EV
accumulation pair of each i-block after the first is deferred into the middle
of the j-loop so the in-order PE never stalls on the previous block's
epilogue reads of the psU banks (which double as the output-projection PSUM).
Output is written bf16 and upcast on host.
"""

import numpy as np
import ml_dtypes

import concourse.bass as bass
import concourse.mybir as mybir
import concourse.tile as tile
from concourse import bacc
from concourse import bass_utils

C = 512          # channels
HW = 4096        # pixels per plane (64*64)
NQ = 2048        # query pixels per core
B = 4            # batch
N_CORES = 8
CT = C // 128    # channel tiles (4)
JT = HW // 128   # key tiles on partitions (32)
JP = JT // 2     # key tile pairs for DoubleRow (16)
IB = NQ // 512   # query i-blocks of 512 (4)
KNB = HW // 512  # key n-chunks for k projection (8)
EPS = 1e-5
SCALE = 1.0 / np.sqrt(np.float32(C))  # 1/sqrt(512)
WS = 16.0        # ws weight scale (q/k/v psum = 16x true)
ALPHA = 4.0      # wp weight scale (o stored as v/4)
BETA = WS * ALPHA  # Z-reduction ones value (cancels WS on V, ALPHA on wp)
EXP_OFF = -4.25  # exp offset; cancels in U/Z, keeps fp8 exp in range

F32 = mybir.dt.float32
BF16 = mybir.dt.bfloat16
FP8 = mybir.dt.float8e4
AF = mybir.ActivationFunctionType
ALU = mybir.AluOpType


def build_nc():
    nc = bacc.Bacc("TRN2", target_bir_lowering=False, debug=False,
                   num_devices=N_CORES)
    # x in fp8 DR pair layout: x8[p, 2g+r, j] = x[(2g+r)*128 + p, j]
    x8 = nc.dram_tensor("x8", [128, CT, HW], FP8, kind="ExternalInput").ap()
    # bf16 residual (query half, raw x)
    xr = nc.dram_tensor("xr", [C, NQ], BF16, kind="ExternalInput").ap()
    # packed weights: w_rs[p, ct*C + o] = w.T[ct*128 + p, o]
    wqT = nc.dram_tensor("wqT", [128, CT * C], BF16, kind="ExternalInput").ap()
    wkT = nc.dram_tensor("wkT", [128, CT * C], BF16, kind="ExternalInput").ap()
    wvT = nc.dram_tensor("wvT", [128, CT * C], BF16, kind="ExternalInput").ap()
    wpT = nc.dram_tensor("wpT", [128, CT * C], BF16, kind="ExternalInput").ap()
    # packed bias constants [128, 12]: cols 0-3 = 16*bq, 4-7 = 16*bk,
    # 8-11 = wp@bv + bp, laid out b[ct*128 + p] -> [p, ct]
    bcons = nc.dram_tensor("bcons", [128, 3 * CT], F32,
                           kind="ExternalInput").ap()
    out = nc.dram_tensor("out", [C, NQ], BF16, kind="ExternalOutput").ap()

    with tile.TileContext(nc) as tc:
        build_graph(tc, x8, xr, wqT, wkT, wvT, wpT, bcons, out)
    nc.compile()
    return nc


def build_graph(tc, x8, xr, wqT, wkT, wvT, wpT, bcons, out):
    nc = tc.nc
    DR = mybir.MatmulPerfMode.DoubleRow
    with (
        tc.tile_pool(name="const", bufs=1) as const,
        tc.tile_pool(name="qk", bufs=1) as qkp,
        tc.tile_pool(name="vt", bufs=1) as vtp,
        tc.tile_pool(name="bc", bufs=1) as bcp,
    ):
        # ---- persistent tiles ----
        x_sb = [qkp.tile([128, 2, HW], FP8, tag=f"x{g}", name=f"x{g}")
                for g in range(2)]
        q_sb = [qkp.tile([128, 2, NQ], FP8, tag=f"q{g}", name=f"q{g}")
                for g in range(2)]
        k_sb = [qkp.tile([128, 2, HW], FP8, tag=f"k{g}", name=f"k{g}")
                for g in range(2)]
        vT_sb = [vtp.tile([128, 2, C], FP8, tag=f"vT{jtp}", name=f"vT{jtp}")
                 for jtp in range(JP)]
        ws_sb = {}
        for wname in ("wq", "wk", "wv"):
            ws_sb[wname] = [qkp.tile([128, 2, C], FP8, tag=f"{wname}s{g}",
                                     name=f"{wname}s{g}") for g in range(2)]
        wp4_sb = [qkp.tile([128, 2, C], FP8, tag=f"wp4{g}", name=f"wp4{g}")
                  for g in range(2)]

        # ---- x first, split across both hwdge trigger queues so neither
        # serializes: sync takes the DVE stats planes (0,1,2h1), scalar takes
        # the ACT stats planes (3, 2h2) ----
        x_dma_insts = []
        for ct, h, eng in ((0, 0, nc.sync), (3, 0, nc.scalar),
                           (0, 1, nc.sync), (3, 1, nc.scalar),
                           (1, 0, nc.sync), (2, 0, nc.scalar),
                           (1, 1, nc.sync), (2, 1, nc.scalar)):
            g, r = divmod(ct, 2)
            csl = slice(h * 2048, (h + 1) * 2048)
            x_dma_insts.append(eng.dma_start(
                out=x_sb[g][:, r, csl], in_=x8[:, ct, csl]))
        # weights + bias constants on the sync queue
        w_dma_insts = []
        w_sb = {}
        for wname, wap in (("wq", wqT), ("wk", wkT), ("wv", wvT), ("wp", wpT)):
            t = const.tile([128, CT * C], BF16, tag=wname, name=wname)
            w_dma_insts.append(nc.sync.dma_start(out=t, in_=wap))
            w_sb[wname] = [t[:, ct * C:(ct + 1) * C] for ct in range(CT)]
        bct = const.tile([128, 3 * CT], F32, tag="bcons", name="bcons")
        nc.scalar.dma_start(out=bct, in_=bcons)
        bq_sb = [bct[:, ct:ct + 1] for ct in range(CT)]
        bk_sb = [bct[:, CT + ct:CT + ct + 1] for ct in range(CT)]
        bp2_sb = [bct[:, 2 * CT + ct:2 * CT + ct + 1] for ct in range(CT)]
        ones_sb = const.tile([128, 128], BF16, tag="ones", name="ones")
        nc.vector.memset(ones_sb, BETA)
        eps_sb = const.tile([128, 1], F32, tag="eps", name="eps")
        nc.vector.memset(eps_sb, EPS / (WS * WS))
        expoff_sb = const.tile([128, 1], F32, tag="expoff", name="expoff")
        nc.vector.memset(expoff_sb, EXP_OFF)
        # preload both ACT function tables while ACT waits on x (Square lives
        # in table 0; Sqrt/Exp in table 1 - the lazy load would otherwise put
        # a 1.3us ACT_TABLE_LOAD on the stats critical path)
        dummy_sb = const.tile([128, 1], F32, tag="dummy", name="dummy")
        nc.scalar.activation(out=dummy_sb, in_=eps_sb, func=AF.Sqrt)

        # bias-correction result tiles (written by tiny matmuls below)
        qbias_sb = [bcp.tile([128, 1], F32, tag=f"qb{mt}", name=f"qb{mt}")
                    for mt in range(CT)]
        kbias_sb = [bcp.tile([128, 1], F32, tag=f"kb{mt}", name=f"kb{mt}")
                    for mt in range(CT)]
        # [128, 2, 16] with data in col 0: DoubleRow needs pair stride %16==0
        cvn_sb = [bcp.tile([128, 2, 16], FP8, tag=f"cvn{g}", name=f"cvn{g}")
                  for g in range(2)]
        wpcv_sb = [bcp.tile([128, 1], F32, tag=f"wpcv{mt}", name=f"wpcv{mt}")
                   for mt in range(CT)]
        nmbx_sb = [bcp.tile([128, 2, 16], FP8, tag=f"nmbx{g}", name=f"nmbx{g}")
                   for g in range(2)]

        def q_epi_dst(ct2, nsl):
            return q_sb[ct2 // 2][:, ct2 % 2, nsl]

        def k_epi_dst(ct2, nsl):
            return k_sb[ct2 // 2][:, ct2 % 2, nsl]

        def vt_epi_dst(jt):
            return vT_sb[jt // 2][:, jt % 2, :]

        with (
            tc.tile_pool(name="stat", bufs=4) as stat,
            tc.tile_pool(name="psB", bufs=6, space="PSUM") as psB,
        ):
            # ---- stage A: InstanceNorm stats -> rstd16/nmbx; fp8 weights ----
            # mv4[:, ct, 0] = mu, mv4[:, ct, 1] = var
            mv4 = stat.tile([128, CT, 2], F32, tag="mv4", name="mv4", bufs=1)
            scratch = stat.tile([128, 2048], BF16, tag="scratch",
                                name="scratch", bufs=1)
            # ACT: planes 3 and 2h2 via accumulating Copy (sums) then Square
            # (sum of squares). All Copies first - their data lands first.
            x_p3 = x_sb[1][:, 1, :]
            x_p2 = x_sb[1][:, 0, :]
            sx3 = stat.tile([128, 2], F32, tag="sx3", name="sx3", bufs=1)
            sq3 = stat.tile([128, 2], F32, tag="sq3", name="sq3", bufs=1)
            sxh = stat.tile([128, 2], F32, tag="sxh", name="sxh", bufs=1)
            sc2 = stat.tile([128, 2048], BF16, tag="scratch2",
                            name="scratch2", bufs=1)
            nc.scalar.activation(out=scratch, in_=x_p3[:, :2048], func=AF.Copy,
                                 accum_out=sx3[:, 0:1])
            nc.scalar.activation(out=sc2, in_=x_p3[:, 2048:], func=AF.Copy,
                                 accum_out=sx3[:, 1:2])
            nc.scalar.activation(out=scratch, in_=x_p2[:, 2048:], func=AF.Copy,
                                 accum_out=sxh[:, 0:1])
            nc.scalar.activation(out=scratch, in_=x_p3[:, :2048],
                                 func=AF.Square, accum_out=sq3[:, 0:1])
            nc.scalar.activation(out=sc2, in_=x_p3[:, 2048:], func=AF.Square,
                                 accum_out=sq3[:, 1:2])
            nc.scalar.activation(out=scratch, in_=x_p2[:, 2048:],
                                 func=AF.Square, accum_out=sxh[:, 1:2])
            # DVE: planes 0,1 and the first half of 2 via bn_stats
            for ct in (0, 1):
                g, r = divmod(ct, 2)
                x_pl = x_sb[g][:, r, :]
                stats = stat.tile([128, 8, 6], F32, tag="stats",
                                  name=f"stats{ct}", bufs=2)
                for sg in range(8):
                    nc.vector.bn_stats(out=stats[:, sg, :],
                                       in_=x_pl[:, sg * 512:(sg + 1) * 512])
                nc.vector.bn_aggr(out=mv4[:, ct, :], in_=stats)
            st2 = stat.tile([128, 4, 6], F32, tag="stats", name="stats2h1",
                            bufs=2)
            for sg in range(4):
                nc.vector.bn_stats(out=st2[:, sg, :],
                                   in_=x_p2[:, sg * 512:(sg + 1) * 512])
            mvh = stat.tile([128, 2], F32, tag="mvh", name="mvh", bufs=1)
            nc.vector.bn_aggr(out=mvh, in_=st2)
            # combine plane 2: mu = (mean1 + sx/2048)/2,
            # var = (var1+mean1^2+sq/2048)/2 - mu^2
            cmb = stat.tile([128, 4], F32, tag="cmb", name="cmb", bufs=1)
            nc.vector.tensor_scalar_mul(cmb[:, 0:2], sxh, 1.0 / 2048.0)
            nc.vector.tensor_mul(cmb[:, 2:3], mvh[:, 0:1], mvh[:, 0:1])
            nc.vector.tensor_add(cmb[:, 3:4], mvh[:, 1:2], cmb[:, 2:3])
            nc.vector.tensor_add(cmb[:, 2:3], mvh[:, 0:1], cmb[:, 0:1])
            nc.vector.tensor_scalar_mul(mv4[:, 2, 0:1], cmb[:, 2:3], 0.5)
            nc.vector.tensor_add(cmb[:, 3:4], cmb[:, 3:4], cmb[:, 1:2])
            nc.vector.tensor_scalar_mul(cmb[:, 3:4], cmb[:, 3:4], 0.5)
            mu2sq = stat.tile([128, 1], F32, tag="mu2sq", name="mu2sq", bufs=1)
            nc.vector.tensor_mul(mu2sq, mv4[:, 2, 0:1], mv4[:, 2, 0:1])
            nc.vector.tensor_sub(mv4[:, 2, 1:2], cmb[:, 3:4], mu2sq)
            # combine plane 3
            s3 = stat.tile([128, 2], F32, tag="s3", name="s3", bufs=1)
            nc.vector.tensor_add(s3[:, 0:1], sx3[:, 0:1], sx3[:, 1:2])
            nc.vector.tensor_add(s3[:, 1:2], sq3[:, 0:1], sq3[:, 1:2])
            nc.vector.tensor_scalar_mul(mv4[:, 3, 0:1], s3[:, 0:1], 1.0 / HW)
            m3sq = stat.tile([128, 2], F32, tag="m3sq", name="m3sq", bufs=1)
            nc.vector.tensor_mul(m3sq[:, 0:1], mv4[:, 3, 0:1], mv4[:, 3, 0:1])
            nc.vector.tensor_scalar_mul(m3sq[:, 1:2], s3[:, 1:2], 1.0 / HW)
            nc.vector.tensor_sub(mv4[:, 3, 1:2], m3sq[:, 1:2], m3sq[:, 0:1])

            # rstd16 = 16/sqrt(var+eps), one packed op for all 4 ct
            std4 = stat.tile([128, CT], F32, tag="std4", name="std4", bufs=1)
            nc.scalar.activation(out=std4, in_=mv4[:, :, 1], func=AF.Sqrt,
                                 bias=eps_sb, scale=1.0 / (WS * WS))
            rstd4 = stat.tile([128, CT], F32, tag="rstd4", name="rstd4",
                              bufs=1)
            nc.vector.reciprocal(out=rstd4, in_=std4)
            # nmbx[g][:, r, 0] = -16*mu[2g+r] (fp8, feeds bias-corr matmuls)
            for g in range(2):
                nc.vector.tensor_scalar_mul(nmbx_sb[g][:, :, 0:1],
                                            mv4[:, 2 * g:2 * g + 2, 0:1],
                                            -WS)
            # warm the Exp table set (sqrt and exp live in different table
            # sets; its load would otherwise insert right before the first
            # attention exp and stall the QK->exp pipeline start by ~1.5us).
            # Reading std4 orders this AFTER the real Sqrt - the engine's
            # wait-queue lets ready instructions bypass blocked ones, so a
            # dep-free dummy would run early and get its table re-evicted.
            nc.scalar.activation(out=dummy_sb, in_=std4[:, 0:1], func=AF.Exp)

            # keep weights off the DMA queues until x has landed - they
            # otherwise steal HBM bandwidth from the startup-critical load
            for wi in w_dma_insts:
                bass._add_dep_helper(wi.ins, x_dma_insts[-1].ins, sync=True,
                                     reason="x load first")

            # fp8 DR weight tiles: ws[g][:, r, :] = w_sb[2g+r] * rstd16
            # (all on DVE - gpsimd tensor ops measure ~7.4us each on HW)
            for wname in ("wq", "wk", "wv"):
                for ct in range(CT):
                    g, r = divmod(ct, 2)
                    nc.vector.tensor_scalar_mul(ws_sb[wname][g][:, r, :],
                                                w_sb[wname][ct],
                                                rstd4[:, ct:ct + 1])
            for ct in range(CT):
                g, r = divmod(ct, 2)
                nc.vector.tensor_scalar_mul(wp4_sb[g][:, r, :],
                                            w_sb["wp"][ct], ALPHA)

            # ---- stage B: fp8 DR projections on x8 ----
            # Main matmul groups first, tiny bias-corr matmuls next, epilogues
            # last (epilogues read the bias tiles; the bias matmuls depend on
            # stats from every channel tile and would stall the PE if first).
            def corr_mms(wsname, dst_ps):
                # dst_ps[mt] = sum_c ws[c, mt*128..]* (-16*mu[c])
                for mt in range(CT):
                    psb = psB.tile([128, 1], F32, tag="psBb", bufs=2,
                                   name=f"ps_{wsname}c{mt}")
                    for g in range(2):
                        nc.tensor.matmul(
                            psb,
                            ws_sb[wsname][g][:, :, mt * 128:(mt + 1) * 128],
                            nmbx_sb[g][:, :, 0:1], start=(g == 0),
                            stop=(g == 1), perf_mode=DR)
                    dst_ps.append(psb)

            # q[ct2][:, n*512...] (only first NQ pixels)
            q_ps = {}
            for ct2 in range(CT):
                for n in range(IB):
                    nsl = slice(n * 512, (n + 1) * 512)
                    ps = psB.tile([128, 512], F32, tag="psB",
                                  name=f"psq{ct2}_{n}")
                    for g in range(2):
                        nc.tensor.matmul(
                            ps, ws_sb["wq"][g][:, :, ct2 * 128:(ct2 + 1) * 128],
                            x_sb[g][:, :, nsl],
                            start=(g == 0), stop=(g == 1), perf_mode=DR)
                    q_ps[(ct2, n)] = ps
            qc_ps = []
            corr_mms("wq", qc_ps)
            for mt in range(CT):
                # qbias = psb/16 + 16*bq   (psb = 256*(wq@nmb_true))
                nc.scalar.activation(out=qbias_sb[mt], in_=qc_ps[mt],
                                     func=AF.Identity, bias=bq_sb[mt],
                                     scale=1.0 / WS)
            for (ct2, n), ps in q_ps.items():
                nsl = slice(n * 512, (n + 1) * 512)
                nc.scalar.activation(
                    out=q_epi_dst(ct2, nsl), in_=ps, func=AF.Identity,
                    bias=qbias_sb[ct2], scale=1.0)
            # k[ct2] over all HW pixels
            k_ps = {}
            for ct2 in range(CT):
                for n in range(KNB):
                    nsl = slice(n * 512, (n + 1) * 512)
                    ps = psB.tile([128, 512], F32, tag="psB",
                                  name=f"psk{ct2}_{n}")
                    for g in range(2):
                        nc.tensor.matmul(
                            ps, ws_sb["wk"][g][:, :, ct2 * 128:(ct2 + 1) * 128],
                            x_sb[g][:, :, nsl],
                            start=(g == 0), stop=(g == 1), perf_mode=DR)
                    k_ps[(ct2, n)] = ps
            kc_ps = []
            corr_mms("wk", kc_ps)
            for mt in range(CT):
                nc.scalar.activation(out=kbias_sb[mt], in_=kc_ps[mt],
                                     func=AF.Identity, bias=bk_sb[mt],
                                     scale=1.0 / WS)
            for (ct2, n), ps in k_ps.items():
                nsl = slice(n * 512, (n + 1) * 512)
                if n % 2 == 0:
                    nc.scalar.activation(
                        out=k_epi_dst(ct2, nsl), in_=ps, func=AF.Identity,
                        bias=kbias_sb[ct2], scale=1.0)
                else:
                    nc.vector.tensor_scalar_add(k_epi_dst(ct2, nsl), ps,
                                                kbias_sb[ct2])
            # vT[jt] = [j=128, c=512] at 16x; v bias/const handled downstream
            for jt in range(JT):
                ps = psB.tile([128, 512], F32, tag="psB", name=f"psv{jt}")
                for g in range(2):
                    nc.tensor.matmul(
                        ps, x_sb[g][:, :, jt * 128:(jt + 1) * 128],
                        ws_sb["wv"][g],
                        start=(g == 0), stop=(g == 1), perf_mode=DR)
                if jt % 2 == 0:
                    nc.vector.tensor_copy(vt_epi_dst(jt), ps)
                else:
                    nc.scalar.activation(out=vt_epi_dst(jt), in_=ps,
                                         func=AF.Copy)
            # cvn = 128*(wv@nmb_true) as fp8 DR vector (= psb/2), then
            # wpcv = wp@cvn + bp2 via fp8 DR matmuls (psb = 512*wpcv)
            vc_ps = []
            corr_mms("wv", vc_ps)
            for mt in range(CT):
                # cvn8 = 128*(wv@nmb_true) = psb/2
                nc.scalar.activation(out=cvn_sb[mt // 2][:, mt % 2, 0:1],
                                     in_=vc_ps[mt], func=AF.Copy, scale=0.5)
            for mt in range(CT):
                psb = psB.tile([128, 1], F32, tag="psBb", bufs=2,
                               name=f"pswpcv{mt}")
                for g in range(2):
                    nc.tensor.matmul(
                        psb, wp4_sb[g][:, :, mt * 128:(mt + 1) * 128],
                        cvn_sb[g][:, :, 0:1], start=(g == 0), stop=(g == 1),
                        perf_mode=DR)
                nc.scalar.activation(out=wpcv_sb[mt], in_=psb,
                                     func=AF.Identity,
                                     bias=bp2_sb[mt], scale=2.0 / (256 * ALPHA))

        # ---- stage C: attention + output projection, per i-block ----
        with (
            tc.tile_pool(name="xres", bufs=16) as xresp,
            tc.tile_pool(name="expp", bufs=3) as expp,
            tc.tile_pool(name="exp0p", bufs=2) as exp0p,
            tc.tile_pool(name="op", bufs=2) as op,
            tc.tile_pool(name="yp", bufs=3) as yp,
            tc.tile_pool(name="rzp", bufs=2) as rzp,
            tc.tile_pool(name="zaccp", bufs=2) as zaccp,
            tc.tile_pool(name="psL", bufs=2, space="PSUM") as psLp,
            tc.tile_pool(name="psAcc", bufs=1, space="PSUM") as psAccp,
        ):
            first_exp_inst = None
            for ib in range(IB):
                isl = slice(ib * 512, (ib + 1) * 512)
                # defer the jtp=0 EV accumulation to the end of the j-loop on
                # later i-blocks: the EV chain then first writes psU ~2.7us
                # into the block, past the WAR on the previous block's
                # epilogue reads of the same banks (in-order PE would stall)
                defer = ib > 0
                psU = [psAccp.tile([128, 512], F32, tag=f"psU{ct}",
                                   name=f"psU{ct}_{ib}") for ct in range(CT)]
                zacc = zaccp.tile([128, 512], F32, tag="zacc", name=f"zacc{ib}")
                zaccb = zaccp.tile([128, 512], BF16, tag="zaccb",
                                   name=f"zaccb{ib}")

                # psL pair tiles (2 PSUM banks) -> one exp instruction per
                # pair. Software-pipelined: QK(jtp+1) issues before U(jtp).
                psL_pairs = [None] * JP
                exp_pairs = [None] * JP

                def emit_qk(jtp):
                    ps = psLp.tile([128, 2, 512], F32, tag="psL",
                                   name=f"psL{jtp}_{ib}")
                    for r in range(2):
                        jt = 2 * jtp + r
                        for g in range(2):
                            nc.tensor.matmul(
                                ps[:, r, :],
                                k_sb[g][:, :, jt * 128:(jt + 1) * 128],
                                q_sb[g][:, :, isl],
                                start=(g == 0), stop=(g == 1), perf_mode=DR)
                    psL_pairs[jtp] = ps

                def emit_ev(jtp, start, stop):
                    for ct in range(CT):
                        nc.tensor.matmul(
                            psU[ct], vT_sb[jtp][:, :, ct * 128:(ct + 1) * 128],
                            exp_pairs[jtp], start=start, stop=stop,
                            perf_mode=DR)

                emit_qk(0)
                for jtp in range(JP):
                    if jtp == 0 and defer:
                        ep = exp0p.tile([128, 2, 512], FP8, tag="expT0",
                                        name=f"expT0_{ib}")
                    else:
                        ep = expp.tile([128, 2, 512], FP8, tag="expT",
                                       name=f"expT{jtp}_{ib}")
                    exp_pairs[jtp] = ep
                    einst = nc.scalar.activation(
                        out=ep, in_=psL_pairs[jtp], func=AF.Exp,
                        bias=expoff_sb, scale=float(SCALE / (WS * WS)))
                    if first_exp_inst is None:
                        first_exp_inst = einst
                    if jtp + 1 < JP:
                        emit_qk(jtp + 1)
                    # Z reduction on DVE (half-pair granularity)
                    for r in range(2):
                        jt = 2 * jtp + r
                        exp_dst = ep[:, r, :]
                        if jt == 0:
                            nc.vector.tensor_copy(zacc, exp_dst)
                        elif jt == JT - 1:
                            nc.vector.tensor_add(zaccb, zacc, exp_dst)
                        else:
                            nc.vector.tensor_add(zacc, zacc, exp_dst)
                    if jtp > 0 or not defer:
                        emit_ev(jtp, start=(jtp == (1 if defer else 0)),
                                stop=(jtp == JP - 1))
                    if defer and jtp == 3:
                        # deferred pair-0 accumulation: far enough in that the
                        # previous block's epilogue has released the psU
                        # banks, and off this block's epilogue critical path
                        emit_ev(0, start=False, stop=False)

                # Z partition-reduce + broadcast in one bf16 matmul; psZ rides
                # the psL pair rotation (uses one of its two banks briefly)
                psZp = psLp.tile([128, 2, 512], F32, tag="psL",
                                 name=f"psZ{ib}")
                nc.tensor.matmul(psZp[:, 0, :], ones_sb, zaccb, start=True,
                                 stop=True)
                rzb = rzp.tile([128, 512], F32, tag="rzb", name=f"rzb{ib}")
                nc.vector.reciprocal_approx_fast(out=rzb, in_=psZp[:, 0, :])

                # normalize U before the projection -> fp8 o in DR layout
                o_sb = [op.tile([128, 2, 512], FP8, tag=f"o{g}",
                                name=f"o{g}_{ib}") for g in range(2)]
                for ct in range(CT):
                    nc.vector.tensor_mul(o_sb[ct // 2][:, ct % 2, :],
                                         psU[ct], rzb)

                # output projection (fp8 DR), then y = psP + wpcv_tot + xr.
                # psP reuses psU's bank for this mt: the WAR dependency (the
                # o normalize reading psU[mt]) coincides with psP's own data
                # dependency on o, so the in-order PE never stalls on it.
                for mt in range(CT):
                    psP = psAccp.tile([128, 512], F32, tag=f"psU{mt}",
                                      name=f"psP{mt}_{ib}")
                    for g in range(2):
                        nc.tensor.matmul(
                            psP, wp4_sb[g][:, :, mt * 128:(mt + 1) * 128],
                            o_sb[g],
                            start=(g == 0), stop=(g == 1), perf_mode=DR)
                    xrt = xresp.tile([128, 512], BF16, tag="xr",
                                     name=f"xr{mt}_{ib}")
                    xr_dma = nc.sync.dma_start(
                        out=xrt, in_=xr[mt * 128:(mt + 1) * 128, isl])
                    # keep the residual loads off the DMA queues until the
                    # attention stage is underway - they'd otherwise compete
                    # with the startup x load for HBM bandwidth
                    bass._add_dep_helper(xr_dma.ins, first_exp_inst.ins,
                                         sync=True,
                                         reason="delay residual load")
                    y = yp.tile([128, 512], BF16, tag="y", name=f"y{mt}_{ib}")
                    nc.vector.scalar_tensor_tensor(
                        out=y, in0=psP, scalar=wpcv_sb[mt], in1=xrt,
                        op0=ALU.add, op1=ALU.add)
                    # last block's output triggers ride the (by then idle)
                    # ACT queue - the sync queue's ~600ns/trigger would
                    # serialize into the kernel tail
                    oeng = nc.scalar if ib == IB - 1 else nc.sync
                    oeng.dma_start(out=out[mt * 128:(mt + 1) * 128, isl],
                                   in_=y)


_NC = None


def _get_nc():
    global _NC
    if _NC is None:
        _NC = build_nc()
    return _NC


def make_in_maps(x, wq, bq, wk, bk, wv, bv, wp, bp):
    x = np.asarray(x, dtype=np.float32)
    wq, wk, wv, wp = (np.asarray(a, dtype=np.float32) for a in (wq, wk, wv, wp))
    bq, bk, bv, bp = (np.asarray(a, dtype=np.float32) for a in (bq, bk, bv, bp))
    bp2 = wp @ bv + bp

    def pack_w(w):
        # [p, ct*C + o] = w.T[ct*128 + p, o]
        wT = np.ascontiguousarray(w.T)
        return np.ascontiguousarray(
            wT.reshape(CT, 128, C).transpose(1, 0, 2).reshape(128, CT * C)
        ).astype(ml_dtypes.bfloat16)

    def pack_b(b):
        return np.ascontiguousarray(b.reshape(CT, 128).T).astype(np.float32)

    bcons = np.concatenate(
        [pack_b(WS * bq), pack_b(WS * bk), pack_b(bp2)], axis=1)
    shared = {
        "wqT": pack_w(wq), "wkT": pack_w(wk), "wvT": pack_w(wv),
        "wpT": pack_w(wp), "bcons": np.ascontiguousarray(bcons),
    }
    in_maps = []
    for core in range(N_CORES):
        b, h = divmod(core, 2)
        xb = x[b].reshape(C, HW)
        xc = np.ascontiguousarray(np.roll(xb, -h * NQ, axis=1))
        # fp8 DR pair layout: x8[p, ct, j] = xc[ct*128 + p, j]
        x8 = np.ascontiguousarray(
            xc.reshape(CT, 128, HW).transpose(1, 0, 2)
        ).astype(ml_dtypes.float8_e4m3)
        in_maps.append({
            "x8": x8,
            "xr": xc[:, :NQ].astype(ml_dtypes.bfloat16),
            **shared,
        })
    return in_maps


def assemble_out(results):
    out = np.empty((B, C, HW), dtype=np.float32)
    for core in range(N_CORES):
        b, h = divmod(core, 2)
        out[b][:, h * NQ:(h + 1) * NQ] = results[core]["out"].astype(np.float32)
    return out.reshape(B, C, 64, 64)


def kernel(x, wq, bq, wk, bk, wv, bv, wp, bp):
    nc = _get_nc()
    in_maps = make_in_maps(x, wq, bq, wk, bk, wv, bv, wp, bp)
    res = bass_utils.run_bass_kernel_spmd(nc, in_maps,
                                          core_ids=list(range(N_CORES)))
    return assemble_out(res.results)
